# revision 20
# baseline (speedup 1.0000x reference)
"""Trainium2 Bass kernel for nn_Disentangler (gnn_message_passing).

Reference computation per timestamp t (T=16):
  xn   = LayerNorm_E(x[t])                 [16384, 128] -> first 8192 rows used
  tee  = segment_sum(xn[:8192] by node_idx[t])      [50000, 128]
  pool = blockmean_4(tee)                           [50000, 32]
  agg  = mean over basket slots of pool[stacked]    [64, 32]
  out  = LayerNorm_2048(agg.reshape(1, 2048))

Algebraic reformulation (all FP math on x happens on device):
  For token i with node n_i, A[i, j] = (# occurrences of n_i among basket j's
  782 slots) — an integer count matrix derived purely from the two index
  tensors (host-side index preprocessing).  With per-token LN1 stats
  (m_i, r_i = rsqrt(var_i+eps)), q_i[c] = sum_{e in block c} x[i,e]*g1[e],
  sc[c] = sum_block g1, bb[c] = mean_block b1:

    agg[j, c] = (1/782) * [ sum_i A[i,j]*u_i[c]        (u = q * r/4)
                            - sc[c] * sum_i A[i,j]*w_i  (w = m * r/4)
                            + bb[c] * sum_i A[i,j] ]

  i.e. one token-contraction matmul  A^T @ [u | 1 | w]  per timestamp.
  Tokens whose node appears in no basket have A == 0 and are dropped
  host-side (packed token list, ~5.2k of 8192; padded to NT=5632).

Sharding: data-parallel over T (2 timestamps per core, 8 cores).

v3 pipeline notes:
  - x is packed AND transposed host-side: xbt [T_LOC, 128(E), NT] bf16 — the
    E-major tile is a straight DMA load (no xbar-transpose DMA).
  - HWDGE dispatch costs ~0.6us of sequencer time PER DMA, so input DMAs are
    split across BOTH hwdge engines (ACT + SP) and consts are merged into two
    tensors (wstat bf16, cmerge f32 holding cst4|cmix|bc2 slices).
  - Stats matmuls stream 1024-wide chunks, weight-grouped in pairs
    (sq-MMs with the 16-col ssqsel selector, then x-MMs with W33) to
    amortize the ~100ns weight-swap penalty.
  - Square production is split DVE/ACT/GpSimd; the u-mul runs on GpSimd so
    DVE keeps up with PSUM evacuations.
  - finalize(0) overlaps contraction(1); only finalize(1)+LN2 are tail.
"""

import os
import sys

import ml_dtypes
import numpy as np

# ---------------------------------------------------------------- constants
T = 16
TOK = 16384
E = 128
N_NODE = 8192
NUM_NODES = 50000
COMP_LEN = 64   # J baskets
MAX_LEN = 782
COMP_DIM = 32   # C
EPS = 1e-5

N_CORES = 8
T_LOC = T // N_CORES   # 2 timestamps per core

NT = 5376              # packed tokens (max kept = 5237 on the fixed seed;
                       # +139 margin, ~4.3 sigma of the binomial spread)
CH = NT // 128         # 42 token chunks
NSTATP = 48            # stats rows padded to x16 for the xbar DMA transpose
R4S = 0.25 / MAX_LEN   # folded r/4 * 1/max_len scale
SDS = 1.0 / (R4S * R4S)   # sqrt scale so reciprocal(sd') = r * R4S

# stats chunk layout: ten 512-wide chunks + one 256 tail (matmul out must
# fit one 2KB PSUM bank -> max 512 f32 columns)
CL = [(k * 512, 512) for k in range(10)] + [(5120, 256)]
GROUPS = [(0, 2), (2, 2), (4, 2), (6, 2), (8, 2), (10, 1)]  # chunk pairs

_PROGRAM = None
LAST_RESULTS = None    # BassKernelResults of the last run (for test harness)

BF16 = ml_dtypes.bfloat16


def _build_program():
    import concourse.bacc as bacc
    import concourse.bass as bass
    import concourse.mybir as mybir
    import concourse.tile as tile

    f32 = mybir.dt.float32
    bf16 = mybir.dt.bfloat16

    nc = bacc.Bacc("TRN2", target_bir_lowering=False, debug=False,
                   num_devices=N_CORES)

    xbt_d = nc.dram_tensor("xbt", [T_LOC, E, NT], bf16, kind="ExternalInput")
    am_d = nc.dram_tensor("am", [T_LOC, 128, CH, COMP_LEN], bf16,
                          kind="ExternalInput")
    # wstat cols: 0-31 Wg, 32 ones, 33-47 zero, 48-63 ssqsel (col 49 = 1)
    wstat_d = nc.dram_tensor("wstat", [E, 64], bf16, kind="ExternalInput")
    # cmerge cols: 0-127 cst4 (sc|bb|g2|b2), 128-133 cmix
    # (selT0|selT1|eps|eps*SDS|sel2b0|sel2b1), 134-261 bc2 on rows 0-1
    cmerge_d = nc.dram_tensor("cmerge", [128, 262], f32, kind="ExternalInput")
    out_d = nc.dram_tensor("out", [T_LOC, COMP_LEN, COMP_DIM], f32,
                           kind="ExternalOutput")

    with tile.TileContext(nc) as tc:
        with (
            tc.tile_pool(name="const", bufs=1) as cp,
            tc.tile_pool(name="main", bufs=1) as pool,
            tc.tile_pool(name="small", bufs=1) as sp,
            tc.tile_pool(name="ps", bufs=3, space=bass.MemorySpace.PSUM) as psp,
            tc.tile_pool(name="psc", bufs=1, space=bass.MemorySpace.PSUM) as pscp,
            tc.tile_pool(name="psde", bufs=1, space=bass.MemorySpace.PSUM) as psdep,
        ):
            # ---- input DMAs split across both HWDGE dispatchers.
            # ACT queue: x0 piece 1, wstat, x0 piece 2, cmerge.
            xT = [pool.tile([E, NT], bf16, tag=f"xT{t}", name=f"xT{t}")
                  for t in range(2)]
            a_sb = [pool.tile([128, CH, COMP_LEN], bf16, tag=f"A{t}",
                              name=f"A{t}") for t in range(2)]
            nc.scalar.dma_start(xT[0][:, 0:1024], xbt_d.ap()[0, :, 0:1024])

            wstat = cp.tile([E, 64], bf16)
            nc.scalar.dma_start(wstat[:], wstat_d.ap())
            w33 = wstat[:, 0:33]
            wsq = wstat[:, 48:64]

            nc.scalar.dma_start(xT[0][:, 1024:2560],
                                xbt_d.ap()[0, :, 1024:2560])

            cmerge = cp.tile([128, 262], f32)
            cst4 = cmerge[:, 0:128]
            sc = cst4[:, 0:COMP_DIM]
            bb = cst4[:, COMP_DIM:2 * COMP_DIM]
            g2 = cst4[:, 2 * COMP_DIM:3 * COMP_DIM]
            b2 = cst4[:, 3 * COMP_DIM:4 * COMP_DIM]
            selT = cmerge[:, 128:130]
            epsb = cmerge[:, 130:131]
            epsb2 = cmerge[:, 131:132]
            sel2b = cmerge[0:COMP_LEN, 132:134]
            bcast2 = cmerge[0:2, 134:262]

            # SP queue: rest of x0, all of x1 (in pieces), both A tensors.
            nc.sync.dma_start(xT[0][:, 2560:4096], xbt_d.ap()[0, :, 2560:4096])
            nc.sync.dma_start(xT[0][:, 4096:NT], xbt_d.ap()[0, :, 4096:NT])
            nc.sync.dma_start(xT[1][:, 0:2048], xbt_d.ap()[1, :, 0:2048])
            nc.sync.dma_start(xT[1][:, 2048:4096], xbt_d.ap()[1, :, 2048:4096])
            nc.sync.dma_start(xT[1][:, 4096:NT], xbt_d.ap()[1, :, 4096:NT])
            nc.sync.dma_start(a_sb[0][:], am_d.ap()[0])
            nc.sync.dma_start(a_sb[1][:], am_d.ap()[1])
            nc.sync.dma_start(cmerge[:], cmerge_d.ap())

            # PE warmup on a memset tile while the first x piece loads.
            # The sqrt feeding warm[:, 0] pins the sqrt_and_* activation
            # table (which also contains square/copy) before any square, so
            # ACT loads its LUT exactly once.  high_priority keeps the
            # scheduler from deferring this dead-end warmup work.
            with tc.high_priority():
                epsl = cp.tile([128, 1], f32)
                nc.vector.memset(epsl[:], EPS)
                actw = cp.tile([128, 1], f32)
                nc.scalar.activation(actw[:], epsl[:],
                                     mybir.ActivationFunctionType.Sqrt,
                                     bias=epsl[:])
                warm = cp.tile([E, 512], bf16)
                nc.vector.memset(warm[:], 0.5)
                wwarm = cp.tile([E, 16], bf16)
                nc.vector.memset(wwarm[:], 0.25)
                psw = psp.tile([NSTATP, 1024], f32, tag="psA", name="psA")
                for _ in range(9):
                    nc.tensor.matmul(psw[32:NSTATP, 0:512], wwarm[:], warm[:],
                                     start=True, stop=True)

            # per-t working tiles
            sqT = [pool.tile([E, NT], bf16, tag=f"sqT{t}", name=f"sqT{t}")
                   for t in range(2)]
            stats_e = [pool.tile([NSTATP, NT], bf16, tag=f"se{t}",
                                 name=f"se{t}") for t in range(2)]
            stats_tok = [pool.tile([128, CH, NSTATP], bf16, tag=f"st{t}",
                                   name=f"st{t}") for t in range(2)]
            rhs2 = [pool.tile([128, CH, 34], bf16, tag=f"rhs2{t}",
                              name=f"rhs2{t}") for t in range(2)]

            # contraction PSUM: t0 rows 0-63, t1 rows 64-127
            pscc = pscp.tile([128, 34], f32, tag="psC")
            cat2F = sp.tile([128, 2 * COMP_DIM], f32, tag="cat2F")

            def sq_produce_pair(t, p, eng):
                ksl = slice(p * 1024, min((p + 1) * 1024, NT))
                if ksl.start >= NT:
                    return
                if eng == "v":
                    nc.vector.tensor_mul(sqT[t][:, ksl], xT[t][:, ksl],
                                         xT[t][:, ksl])
                else:
                    nc.scalar.square(sqT[t][:, ksl], xT[t][:, ksl])

            # sq-production engine per 1024-pair (both timestamps)
            SQ_ENG = ["v", "v", "v", "a", "v", "a"]

            def stats_group(t, g0, gn, eng):
                """gn chunks in pairs sharing a [48,1024] 2-bank PSUM tile:
                sq-MMs (wsq), then x-MMs (w33), then one evac per pair."""
                tiles = []   # (ps, col_off, k)
                for i, k in enumerate(range(g0, g0 + gn)):
                    off, ln = CL[k]
                    ksl = slice(off, off + ln)
                    if i % 2 == 0:
                        ps = psp.tile([NSTATP, 1024], f32, tag="psA",
                                      name="psA")
                    co = (i % 2) * 512
                    nc.tensor.matmul(ps[32:NSTATP, co:co + ln], wsq,
                                     sqT[t][:, ksl], start=True, stop=True)
                    tiles.append((ps, co, k))
                for ps, co, k in tiles:
                    off, ln = CL[k]
                    ksl = slice(off, off + ln)
                    nc.tensor.matmul(ps[0:33, co:co + ln], w33, xT[t][:, ksl],
                                     start=True, stop=True)
                for i in range(0, gn, 2):
                    ps, _, k = tiles[i]
                    off0, _ = CL[k]
                    width = sum(CL[k + j][1] for j in range(min(2, gn - i)))
                    esl = slice(off0, off0 + width)
                    if eng == "a":
                        nc.scalar.copy(stats_e[t][:, esl], ps[:, 0:width])
                    else:
                        nc.vector.tensor_copy(stats_e[t][:, esl],
                                              ps[:, 0:width])
                    eng = "a" if eng == "v" else "v"

            EVAC_ENG = ["a", "v", "a", "a", "v", "a"]

            def stats_t(t, part, hooks=()):
                # sq pairs are produced one group ahead of their matmuls
                hooks = dict(hooks)
                if part == 0:
                    sq_produce_pair(t, 0, SQ_ENG[0])
                    for gidx in range(3):
                        sq_produce_pair(t, gidx + 1, SQ_ENG[gidx + 1])
                        stats_group(t, *GROUPS[gidx], EVAC_ENG[gidx])
                        if gidx in hooks:
                            hooks[gidx]()
                else:
                    for gidx in range(3, 6):
                        if gidx + 1 < 6:
                            sq_produce_pair(t, gidx + 1, SQ_ENG[gidx + 1])
                        stats_group(t, *GROUPS[gidx], EVAC_ENG[gidx])
                        if gidx in hooks:
                            hooks[gidx]()

            def transpose_part(t, c0, cn):
                nc.sync.dma_start_transpose(
                    stats_tok[t][:, c0:c0 + cn, :],
                    stats_e[t][:, c0 * 128:(c0 + cn) * 128])

            def rhs2_chain(t, c0, cn, tag):
                hsl = slice(c0, c0 + cn)
                st = stats_tok[t]
                m_f = sp.tile([128, cn], f32, tag=f"m{tag}", name=f"m{tag}")
                nc.vector.tensor_scalar_mul(m_f[:], st[:, hsl, 32], 1.0 / E)
                v_f = sp.tile([128, cn], f32, tag=f"v{tag}", name=f"v{tag}")
                nc.vector.tensor_scalar_mul(v_f[:], st[:, hsl, 33], 1.0 / E)
                m2_f = sp.tile([128, cn], f32, tag=f"m2{tag}",
                               name=f"m2{tag}")
                nc.vector.tensor_mul(m2_f[:], m_f[:], m_f[:])
                nc.vector.tensor_sub(v_f[:], v_f[:], m2_f[:])
                # sd' = sqrt(v*SDS + eps*SDS);  r4 = 1/sd' = R4S * rsqrt(v+eps)
                sd_f = sp.tile([128, cn], f32, tag=f"sd{tag}",
                               name=f"sd{tag}")
                nc.scalar.activation(sd_f[:], v_f[:],
                                     mybir.ActivationFunctionType.Sqrt,
                                     bias=epsb2, scale=float(SDS))
                r4_f = sp.tile([128, cn], f32, tag=f"r4{tag}",
                               name=f"r4{tag}")
                nc.vector.reciprocal(r4_f[:], sd_f[:])
                r4_b = sp.tile([128, cn], bf16, tag=f"r4b{tag}",
                               name=f"r4b{tag}")
                nc.vector.tensor_copy(r4_b[:], r4_f[:])
                bcast = r4_b[:].unsqueeze(2).broadcast_to([128, cn, COMP_DIM])
                nc.vector.tensor_mul(rhs2[t][:, hsl, 0:COMP_DIM],
                                     st[:, hsl, 0:COMP_DIM], bcast)
                nc.vector.memset(rhs2[t][:, hsl, 32:33], 1.0)
                nc.vector.tensor_mul(rhs2[t][:, hsl, 33], m_f[:], r4_f[:])

            def contraction(t):
                rows = slice(t * COMP_LEN, (t + 1) * COMP_LEN)
                for g in range(CH):
                    nc.tensor.matmul(pscc[rows, :], a_sb[t][:, g, :],
                                     rhs2[t][:, g, :],
                                     start=(g == 0), stop=(g == CH - 1))

            def finalize(t):
                # cat2F = psC_q + bb*psC[32] - sc*psC[33]  (sc stored negated)
                rows = slice(t * COMP_LEN, (t + 1) * COMP_LEN)
                pscX = pscc[rows, :]
                e1 = sp.tile([128, COMP_DIM], f32, tag=f"e1f{t}",
                             name=f"e1f{t}")
                nc.vector.scalar_tensor_tensor(
                    e1[rows, :], sc[rows, :], pscX[:, 33:34],
                    pscX[:, 0:COMP_DIM],
                    op0=mybir.AluOpType.mult, op1=mybir.AluOpType.add)
                nc.vector.scalar_tensor_tensor(
                    cat2F[rows, 0:COMP_DIM], bb[rows, :], pscX[:, 32:33],
                    e1[rows, :],
                    op0=mybir.AluOpType.mult, op1=mybir.AluOpType.add)
                nc.vector.tensor_mul(cat2F[rows, COMP_DIM:2 * COMP_DIM],
                                     cat2F[rows, 0:COMP_DIM],
                                     cat2F[rows, 0:COMP_DIM])

            # late-needed DMA dispatches, issued from the ACT queue at
            # FIFO positions that delay them past the x0 stream
            def disp_x1b():
                nc.scalar.dma_start(xT[1][:, 2048:4096],
                                    xbt_d.ap()[1, :, 2048:4096])

            def disp_x1c_cm():
                nc.scalar.dma_start(xT[1][:, 4096:NT],
                                    xbt_d.ap()[1, :, 4096:NT])
                nc.scalar.dma_start(cmerge[:], cmerge_d.ap())

            def disp_a0():
                nc.scalar.dma_start(a_sb[0][:], am_d.ap()[0])

            def disp_a1():
                nc.scalar.dma_start(a_sb[1][:], am_d.ap()[1])

            # ---- interleaved schedule: PE never idles long enough to cool
            stats_t(0, 0, hooks={0: disp_x1b, 1: disp_x1c_cm})
            stats_t(0, 1, hooks={3: disp_a0, 4: disp_a1})
            transpose_part(0, 0, 21)
            transpose_part(0, 21, 21)
            stats_t(1, 0)
            rhs2_chain(0, 0, 21, "00")
            rhs2_chain(0, 21, 21, "01")
            stats_t(1, 1)
            contraction(0)
            transpose_part(1, 0, 11)
            transpose_part(1, 11, 11)
            transpose_part(1, 22, 10)
            transpose_part(1, 32, 10)
            rhs2_chain(1, 0, 11, "10")
            rhs2_chain(1, 11, 11, "11")
            rhs2_chain(1, 22, 10, "12")
            rhs2_chain(1, 32, 10, "13")
            finalize(0)
            contraction(1)
            finalize(1)

            # ---- fused LN2 for both timestamps (sel2b carries 1/2048)
            psd = psdep.tile([2 * COMP_DIM, 2], f32, tag="psDE")
            nc.tensor.matmul(psd[:], cat2F[:], selT, start=True, stop=True)
            sD = sp.tile([2 * COMP_DIM, 2], f32, tag="sD")
            nc.vector.tensor_copy(sD[:], psd[:])
            pse = psdep.tile([2, 2], f32, tag="psDE")
            nc.tensor.matmul(pse[:], sD[:], sel2b, start=True, stop=True)
            sE = sp.tile([2, 2], f32, tag="sE")
            nc.vector.tensor_copy(sE[:], pse[:])
            psf = psdep.tile([128, 2], f32, tag="psDE")
            nc.tensor.matmul(psf[:], bcast2, sE[:], start=True, stop=True)
            bS = sp.tile([128, 2], f32, tag="bS")
            nc.vector.tensor_copy(bS[:], psf[:])

            mu = bS[:, 0:1]
            mu2 = sp.tile([128, 1], f32, tag="mu2")
            nc.vector.tensor_mul(mu2[:], bS[:, 0:1], bS[:, 0:1])
            ex2 = sp.tile([128, 1], f32, tag="ex2")
            nc.vector.tensor_sub(ex2[:], bS[:, 1:2], mu2[:])
            sd2 = sp.tile([128, 1], f32, tag="sd2")
            nc.scalar.activation(sd2[:], ex2[:],
                                 mybir.ActivationFunctionType.Sqrt,
                                 bias=epsb)
            rr = sp.tile([128, 1], f32, tag="rr")
            nc.vector.reciprocal(rr[:], sd2[:])

            obuf = sp.tile([128, COMP_DIM], f32, tag="obuf")
            nc.vector.tensor_scalar(obuf[:], cat2F[:, 0:COMP_DIM],
                                    mu, rr[:],
                                    mybir.AluOpType.subtract,
                                    mybir.AluOpType.mult)
            nc.vector.tensor_mul(obuf[:], obuf[:], g2)
            nc.vector.tensor_add(obuf[:], obuf[:], b2)

            nc.sync.dma_start(out_d.ap().rearrange("t j c -> (t j) c"), obuf[:])

    nc.compile()
    return nc


def _get_program():
    global _PROGRAM
    if _PROGRAM is None:
        _PROGRAM = _build_program()
    return _PROGRAM


def _prepare_inputs(x, ln1_g, ln1_b, ln2_g, ln2_b, node_idx, stacked_indices):
    """Host-side index preprocessing + weight prep. Returns list of in_maps."""
    node_idx = np.asarray(node_idx).astype(np.int64)
    stacked = np.asarray(stacked_indices).astype(np.int64)
    x = np.asarray(x, dtype=np.float32)
    ln1_g = np.asarray(ln1_g, dtype=np.float32)
    ln1_b = np.asarray(ln1_b, dtype=np.float32)
    ln2_g = np.asarray(ln2_g, dtype=np.float32)
    ln2_b = np.asarray(ln2_b, dtype=np.float32)

    # histogram bt[n, j] = count of node n in basket j  (index preprocessing)
    bt = np.zeros((NUM_NODES, COMP_LEN), dtype=np.float32)
    j_ids = np.broadcast_to(np.arange(COMP_LEN)[:, None], stacked.shape)
    np.add.at(bt, (stacked.ravel(), j_ids.ravel()), 1.0)
    node_used = bt.any(axis=1)

    # weight prep: [Wg(32) | 1 | 0*15 | ssqsel(16, col 49 = 1)]
    wstat = np.zeros((E, 64), dtype=np.float32)
    wstat[np.arange(E), np.arange(E) // 4] = ln1_g
    wstat[:, 32] = 1.0
    wstat[:, 49] = 1.0
    wstat_bf = wstat.astype(BF16)
    scv = ln1_g.reshape(COMP_DIM, 4).sum(1)
    bbv = ln1_b.reshape(COMP_DIM, 4).mean(1)
    # sc is used against lambda which already carries 1/max_len (via R4S)
    sc782 = np.broadcast_to(-scv, (COMP_LEN, COMP_DIM))
    bb782 = np.broadcast_to(bbv / MAX_LEN, (COMP_LEN, COMP_DIM))
    g2 = ln2_g.reshape(COMP_LEN, COMP_DIM)
    b2 = ln2_b.reshape(COMP_LEN, COMP_DIM)
    cst4 = np.tile(
        np.concatenate([sc782, bb782, g2, b2], axis=1).astype(np.float32),
        (2, 1))

    cmerge = np.zeros((128, 262), dtype=np.float32)
    cmerge[:, 0:128] = cst4
    cmerge[0:COMP_LEN, 128] = 1.0           # selT col 0
    cmerge[COMP_LEN:128, 129] = 1.0         # selT col 1
    cmerge[:, 130] = EPS                    # LN2 sqrt bias
    cmerge[:, 131] = EPS * SDS              # folded LN1 sqrt bias
    cmerge[0:COMP_DIM, 132] = 1.0 / 2048.0  # sel2b col 0
    cmerge[COMP_DIM:COMP_LEN, 133] = 1.0 / 2048.0
    cmerge[0, 134:134 + COMP_LEN] = 1.0     # bc2 row 0
    cmerge[1, 134 + COMP_LEN:262] = 1.0     # bc2 row 1

    in_maps = []
    for core in range(N_CORES):
        ts = list(range(core * T_LOC, (core + 1) * T_LOC))
        am = np.zeros((T_LOC, 128, CH, COMP_LEN), dtype=BF16)
        xbt = np.empty((T_LOC, E, NT), dtype=BF16)
        for ti, tg in enumerate(ts):
            nt_ids = node_idx[tg, :N_NODE]
            kept = np.flatnonzero(node_used[nt_ids])
            if len(kept) > NT:
                print(f"WARNING: kept token overflow {len(kept)} > {NT}",
                      file=sys.stderr)
                kept = kept[:NT]
            nk = len(kept)
            sel = np.zeros(NT, dtype=np.int64)
            sel[:nk] = kept
            xbt[ti] = np.ascontiguousarray(x[tg, sel, :].T).astype(BF16)
            a_full = bt[nt_ids[sel], :]
            a_full[nk:, :] = 0.0
            am[ti] = a_full.reshape(CH, 128, COMP_LEN).transpose(1, 0, 2)
        in_maps.append({
            "xbt": xbt,
            "am": am,
            "wstat": wstat_bf,
            "cmerge": cmerge,
        })
    return in_maps


def kernel(x, ln1_g, ln1_b, ln2_g, ln2_b, node_idx, stacked_indices,
           n_node=N_NODE, num_nodes=NUM_NODES):
    global LAST_RESULTS
    from concourse.bass_utils import run_bass_kernel_spmd

    nc = _get_program()
    in_maps = _prepare_inputs(x, ln1_g, ln1_b, ln2_g, ln2_b, node_idx,
                              stacked_indices)

    if os.environ.get("KERNEL_SIM"):
        outs = _run_sim(nc, in_maps)
    else:
        res = run_bass_kernel_spmd(
            nc, in_maps, core_ids=list(range(N_CORES)),
            trace=bool(os.environ.get("KERNEL_TRACE")),
        )
        LAST_RESULTS = res
        outs = [r["out"] for r in res.results]

    full = np.concatenate(outs, axis=0)           # [16, 64, 32]
    return full.reshape(T, 1, COMP_LEN * COMP_DIM).astype(np.float32)


def _run_sim(nc, in_maps):
    """CoreSim path (KERNEL_SIM=1): simulate cores serially."""
    from concourse.bass_interp import CoreSim
    outs = []
    ncores = int(os.environ.get("KERNEL_SIM_CORES", "1"))
    for core, im in enumerate(in_maps[:ncores]):
        sim = CoreSim(nc, trace=False)
        for k, v in im.items():
            sim.tensor(k)[:] = v
        sim.simulate(check_with_hw=False)
        outs.append(np.array(sim.tensor("out")))
    for core in range(ncores, len(in_maps)):
        outs.append(np.zeros((T_LOC, COMP_LEN, COMP_DIM), np.float32))
    return outs


# revision 21
# speedup vs baseline: 1.1752x; 1.1752x over previous
"""Trainium2 Bass kernel for nn_Disentangler (gnn_message_passing).

Reference computation per timestamp t (T=16):
  xn   = LayerNorm_E(x[t])                 [16384, 128] -> first 8192 rows used
  tee  = segment_sum(xn[:8192] by node_idx[t])      [50000, 128]
  pool = blockmean_4(tee)                           [50000, 32]
  agg  = mean over basket slots of pool[stacked]    [64, 32]
  out  = LayerNorm_2048(agg.reshape(1, 2048))

Algebraic reformulation (all FP math on x happens on device):
  For token i with node n_i, A[i, j] = (# occurrences of n_i among basket j's
  782 slots) — an integer count matrix derived purely from the two index
  tensors (host-side index preprocessing).  With per-token LN1 stats
  (m_i, r_i = rsqrt(var_i+eps)), q_i[c] = sum_{e in block c} x[i,e]*g1[e],
  sc[c] = sum_block g1, bb[c] = mean_block b1:

    agg[j, c] = (1/782) * [ sum_i A[i,j]*u_i[c]        (u = q * r/4)
                            - sc[c] * sum_i A[i,j]*w_i  (w = m * r/4)
                            + bb[c] * sum_i A[i,j] ]

  i.e. one token-contraction matmul  A^T @ [u | 1 | w]  per timestamp.
  Tokens whose node appears in no basket have A == 0 and are dropped
  host-side (packed token list, ~5.2k of 8192; padded to NT=5632).

Sharding: data-parallel over T (2 timestamps per core, 8 cores).

v3 pipeline notes:
  - x is packed AND transposed host-side: xbt [T_LOC, 128(E), NT] bf16 — the
    E-major tile is a straight DMA load (no xbar-transpose DMA).
  - HWDGE dispatch costs ~0.6us of sequencer time PER DMA, so input DMAs are
    split across BOTH hwdge engines (ACT + SP) and consts are merged into two
    tensors (wstat bf16, cmerge f32 holding cst4|cmix|bc2 slices).
  - Stats matmuls stream 1024-wide chunks, weight-grouped in pairs
    (sq-MMs with the 16-col ssqsel selector, then x-MMs with W33) to
    amortize the ~100ns weight-swap penalty.
  - Square production is split DVE/ACT/GpSimd; the u-mul runs on GpSimd so
    DVE keeps up with PSUM evacuations.
  - finalize(0) overlaps contraction(1); only finalize(1)+LN2 are tail.
"""

import os
import sys

import ml_dtypes
import numpy as np

# ---------------------------------------------------------------- constants
T = 16
TOK = 16384
E = 128
N_NODE = 8192
NUM_NODES = 50000
COMP_LEN = 64   # J baskets
MAX_LEN = 782
COMP_DIM = 32   # C
EPS = 1e-5

N_CORES = 8
T_LOC = T // N_CORES   # 2 timestamps per core

NT = 5376              # packed tokens (max kept = 5237 on the fixed seed;
                       # +139 margin, ~4.3 sigma of the binomial spread)
CH = NT // 128         # 42 token chunks
NSTATP = 48            # stats rows padded to x16 for the xbar DMA transpose
R4S = 0.25 / MAX_LEN   # folded r/4 * 1/max_len scale
SDS = 1.0 / (R4S * R4S)   # sqrt scale so reciprocal(sd') = r * R4S

# stats chunk layout: ten 512-wide chunks + one 256 tail (matmul out must
# fit one 2KB PSUM bank -> max 512 f32 columns)
CL = [(k * 512, 512) for k in range(10)] + [(5120, 256)]
GROUPS = [(0, 2), (2, 2), (4, 2), (6, 2), (8, 2), (10, 1)]  # chunk pairs

_PROGRAM = None
LAST_RESULTS = None    # BassKernelResults of the last run (for test harness)

BF16 = ml_dtypes.bfloat16


def _build_program():
    import concourse.bacc as bacc
    import concourse.bass as bass
    import concourse.mybir as mybir
    import concourse.tile as tile

    f32 = mybir.dt.float32
    bf16 = mybir.dt.bfloat16

    nc = bacc.Bacc("TRN2", target_bir_lowering=False, debug=False,
                   num_devices=N_CORES)

    xbt_d = nc.dram_tensor("xbt", [T_LOC, E, NT], bf16, kind="ExternalInput")
    f8 = mybir.dt.float8e4
    am_d = nc.dram_tensor("am", [T_LOC, 128, CH, COMP_LEN], f8,
                          kind="ExternalInput")
    # wstat cols: 0-31 Wg, 32 ones, 33-47 zero, 48-63 ssqsel (col 49 = 1)
    wstat_d = nc.dram_tensor("wstat", [E, 64], bf16, kind="ExternalInput")
    # cmerge cols: 0-127 cst4 (sc|bb|g2|b2), 128-133 cmix
    # (selT0|selT1|eps|eps*SDS|sel2b0|sel2b1), 134-261 bc2 on rows 0-1
    cmerge_d = nc.dram_tensor("cmerge", [128, 262], f32, kind="ExternalInput")
    out_d = nc.dram_tensor("out", [T_LOC, COMP_LEN, COMP_DIM], f32,
                           kind="ExternalOutput")

    with tile.TileContext(nc) as tc:
        with (
            tc.tile_pool(name="const", bufs=1) as cp,
            tc.tile_pool(name="main", bufs=1) as pool,
            tc.tile_pool(name="small", bufs=1) as sp,
            tc.tile_pool(name="ps", bufs=3, space=bass.MemorySpace.PSUM) as psp,
            tc.tile_pool(name="psc", bufs=1, space=bass.MemorySpace.PSUM) as pscp,
            tc.tile_pool(name="psde", bufs=1, space=bass.MemorySpace.PSUM) as psdep,
        ):
            # ---- input DMAs split across both HWDGE dispatchers.
            # ACT queue: x0 piece 1, wstat, x0 piece 2, cmerge.
            xT = [pool.tile([E, NT], bf16, tag=f"xT{t}", name=f"xT{t}")
                  for t in range(2)]
            a_sb = [pool.tile([128, CH, COMP_LEN], f8, tag=f"A{t}",
                              name=f"A{t}") for t in range(2)]
            nc.scalar.dma_start(xT[0][:, 0:1024], xbt_d.ap()[0, :, 0:1024])

            wstat = cp.tile([E, 64], bf16)
            nc.scalar.dma_start(wstat[:], wstat_d.ap())
            w33 = wstat[:, 0:33]
            wsq = wstat[:, 48:64]

            nc.scalar.dma_start(xT[0][:, 1024:2560],
                                xbt_d.ap()[0, :, 1024:2560])

            cmerge = cp.tile([128, 262], f32)
            cst4 = cmerge[:, 0:128]
            sc = cst4[:, 0:COMP_DIM]
            bb = cst4[:, COMP_DIM:2 * COMP_DIM]
            g2 = cst4[:, 2 * COMP_DIM:3 * COMP_DIM]
            b2 = cst4[:, 3 * COMP_DIM:4 * COMP_DIM]
            selT = cmerge[:, 128:130]
            epsb = cmerge[:, 130:131]
            epsb2 = cmerge[:, 131:132]
            sel2b = cmerge[0:COMP_LEN, 132:134]
            bcast2 = cmerge[0:2, 134:262]

            # SP queue: rest of x0, all of x1 (in pieces), both A tensors.
            nc.sync.dma_start(xT[0][:, 2560:4096], xbt_d.ap()[0, :, 2560:4096])
            nc.sync.dma_start(xT[0][:, 4096:NT], xbt_d.ap()[0, :, 4096:NT])
            nc.sync.dma_start(xT[1][:, 0:2048], xbt_d.ap()[1, :, 0:2048])
            nc.sync.dma_start(xT[1][:, 2048:4096], xbt_d.ap()[1, :, 2048:4096])
            nc.sync.dma_start(xT[1][:, 4096:NT], xbt_d.ap()[1, :, 4096:NT])
            nc.sync.dma_start(a_sb[0][:], am_d.ap()[0])
            nc.sync.dma_start(a_sb[1][:], am_d.ap()[1])
            nc.sync.dma_start(cmerge[:], cmerge_d.ap())

            # PE warmup on a memset tile while the first x piece loads.
            # The sqrt feeding warm[:, 0] pins the sqrt_and_* activation
            # table (which also contains square/copy) before any square, so
            # ACT loads its LUT exactly once.  high_priority keeps the
            # scheduler from deferring this dead-end warmup work.
            with tc.high_priority():
                epsl = cp.tile([128, 1], f32)
                nc.vector.memset(epsl[:], EPS)
                actw = cp.tile([128, 1], f32)
                nc.scalar.activation(actw[:], epsl[:],
                                     mybir.ActivationFunctionType.Sqrt,
                                     bias=epsl[:])
                warm = cp.tile([E, 512], bf16)
                nc.vector.memset(warm[:], 0.5)
                wwarm = cp.tile([E, 16], bf16)
                nc.vector.memset(wwarm[:], 0.25)
                psw = psp.tile([NSTATP, 1024], f32, tag="psA", name="psA")
                for _ in range(9):
                    nc.tensor.matmul(psw[32:NSTATP, 0:512], wwarm[:], warm[:],
                                     start=True, stop=True)

            # per-t working tiles
            sqT = [pool.tile([E, NT], bf16, tag=f"sqT{t}", name=f"sqT{t}")
                   for t in range(2)]
            stats_e = [pool.tile([NSTATP, NT], bf16, tag=f"se{t}",
                                 name=f"se{t}") for t in range(2)]
            stats_tok = [pool.tile([128, CH, NSTATP], bf16, tag=f"st{t}",
                                   name=f"st{t}") for t in range(2)]
            rhs2 = [pool.tile([128, CH, 34], bf16, tag=f"rhs2{t}",
                              name=f"rhs2{t}") for t in range(2)]

            # contraction PSUM: t0 rows 0-63, t1 rows 64-127
            pscc = pscp.tile([128, 34], f32, tag="psC")
            cat2F = sp.tile([128, 2 * COMP_DIM], f32, tag="cat2F")

            def sq_produce_pair(t, p, eng):
                ksl = slice(p * 1024, min((p + 1) * 1024, NT))
                if ksl.start >= NT:
                    return
                if eng == "v":
                    nc.vector.tensor_mul(sqT[t][:, ksl], xT[t][:, ksl],
                                         xT[t][:, ksl])
                else:
                    nc.scalar.square(sqT[t][:, ksl], xT[t][:, ksl])

            # sq-production engine per 1024-pair (both timestamps)
            SQ_ENG = ["v", "v", "v", "a", "v", "a"]

            def stats_group(t, g0, gn, eng):
                """gn chunks in pairs sharing a [48,1024] 2-bank PSUM tile:
                sq-MMs (wsq), then x-MMs (w33), then one evac per pair."""
                tiles = []   # (ps, col_off, k)
                for i, k in enumerate(range(g0, g0 + gn)):
                    off, ln = CL[k]
                    ksl = slice(off, off + ln)
                    if i % 2 == 0:
                        ps = psp.tile([NSTATP, 1024], f32, tag="psA",
                                      name="psA")
                    co = (i % 2) * 512
                    nc.tensor.matmul(ps[32:NSTATP, co:co + ln], wsq,
                                     sqT[t][:, ksl], start=True, stop=True)
                    tiles.append((ps, co, k))
                for ps, co, k in tiles:
                    off, ln = CL[k]
                    ksl = slice(off, off + ln)
                    nc.tensor.matmul(ps[0:33, co:co + ln], w33, xT[t][:, ksl],
                                     start=True, stop=True)
                for i in range(0, gn, 2):
                    ps, _, k = tiles[i]
                    off0, _ = CL[k]
                    width = sum(CL[k + j][1] for j in range(min(2, gn - i)))
                    esl = slice(off0, off0 + width)
                    if eng == "a":
                        nc.scalar.copy(stats_e[t][:, esl], ps[:, 0:width])
                    else:
                        nc.vector.tensor_copy(stats_e[t][:, esl],
                                              ps[:, 0:width])
                    eng = "a" if eng == "v" else "v"

            EVAC_ENG = ["a", "v", "a", "a", "v", "a"]

            def stats_t(t, part, hooks=()):
                # sq pairs are produced one group ahead of their matmuls
                hooks = dict(hooks)
                if part == 0:
                    sq_produce_pair(t, 0, SQ_ENG[0])
                    for gidx in range(3):
                        sq_produce_pair(t, gidx + 1, SQ_ENG[gidx + 1])
                        stats_group(t, *GROUPS[gidx], EVAC_ENG[gidx])
                        if gidx in hooks:
                            hooks[gidx]()
                else:
                    for gidx in range(3, 6):
                        if gidx + 1 < 6:
                            sq_produce_pair(t, gidx + 1, SQ_ENG[gidx + 1])
                        stats_group(t, *GROUPS[gidx], EVAC_ENG[gidx])
                        if gidx in hooks:
                            hooks[gidx]()

            def transpose_part(t, c0, cn):
                nc.sync.dma_start_transpose(
                    stats_tok[t][:, c0:c0 + cn, :],
                    stats_e[t][:, c0 * 128:(c0 + cn) * 128])

            def rhs2_chain(t, c0, cn, tag):
                hsl = slice(c0, c0 + cn)
                st = stats_tok[t]
                m_f = sp.tile([128, cn], f32, tag=f"m{tag}", name=f"m{tag}")
                nc.vector.tensor_scalar_mul(m_f[:], st[:, hsl, 32], 1.0 / E)
                v_f = sp.tile([128, cn], f32, tag=f"v{tag}", name=f"v{tag}")
                nc.vector.tensor_scalar_mul(v_f[:], st[:, hsl, 33], 1.0 / E)
                m2_f = sp.tile([128, cn], f32, tag=f"m2{tag}",
                               name=f"m2{tag}")
                nc.vector.tensor_mul(m2_f[:], m_f[:], m_f[:])
                nc.vector.tensor_sub(v_f[:], v_f[:], m2_f[:])
                # sd' = sqrt(v*SDS + eps*SDS);  r4 = 1/sd' = R4S * rsqrt(v+eps)
                sd_f = sp.tile([128, cn], f32, tag=f"sd{tag}",
                               name=f"sd{tag}")
                nc.scalar.activation(sd_f[:], v_f[:],
                                     mybir.ActivationFunctionType.Sqrt,
                                     bias=epsb2, scale=float(SDS))
                r4_f = sp.tile([128, cn], f32, tag=f"r4{tag}",
                               name=f"r4{tag}")
                nc.vector.reciprocal(r4_f[:], sd_f[:])
                r4_b = sp.tile([128, cn], bf16, tag=f"r4b{tag}",
                               name=f"r4b{tag}")
                nc.vector.tensor_copy(r4_b[:], r4_f[:])
                bcast = r4_b[:].unsqueeze(2).broadcast_to([128, cn, COMP_DIM])
                nc.vector.tensor_mul(rhs2[t][:, hsl, 0:COMP_DIM],
                                     st[:, hsl, 0:COMP_DIM], bcast)
                nc.vector.memset(rhs2[t][:, hsl, 32:33], 1.0)
                nc.vector.tensor_mul(rhs2[t][:, hsl, 33], m_f[:], r4_f[:])

            def contraction(t):
                rows = slice(t * COMP_LEN, (t + 1) * COMP_LEN)
                for g in range(CH):
                    nc.tensor.matmul(pscc[rows, :], a_sb[t][:, g, :],
                                     rhs2[t][:, g, :],
                                     start=(g == 0), stop=(g == CH - 1))

            def finalize(t):
                # cat2F = psC_q + bb*psC[32] - sc*psC[33]  (sc stored negated)
                rows = slice(t * COMP_LEN, (t + 1) * COMP_LEN)
                pscX = pscc[rows, :]
                e1 = sp.tile([128, COMP_DIM], f32, tag=f"e1f{t}",
                             name=f"e1f{t}")
                nc.vector.scalar_tensor_tensor(
                    e1[rows, :], sc[rows, :], pscX[:, 33:34],
                    pscX[:, 0:COMP_DIM],
                    op0=mybir.AluOpType.mult, op1=mybir.AluOpType.add)
                nc.vector.scalar_tensor_tensor(
                    cat2F[rows, 0:COMP_DIM], bb[rows, :], pscX[:, 32:33],
                    e1[rows, :],
                    op0=mybir.AluOpType.mult, op1=mybir.AluOpType.add)
                nc.vector.tensor_mul(cat2F[rows, COMP_DIM:2 * COMP_DIM],
                                     cat2F[rows, 0:COMP_DIM],
                                     cat2F[rows, 0:COMP_DIM])

            # ---- interleaved schedule: PE never idles long enough to cool
            stats_t(0, 0)
            stats_t(0, 1)
            transpose_part(0, 0, 21)
            transpose_part(0, 21, 21)
            stats_t(1, 0)
            rhs2_chain(0, 0, 21, "00")
            rhs2_chain(0, 21, 21, "01")
            stats_t(1, 1)
            contraction(0)
            transpose_part(1, 0, 11)
            transpose_part(1, 11, 11)
            transpose_part(1, 22, 10)
            transpose_part(1, 32, 10)
            rhs2_chain(1, 0, 11, "10")
            rhs2_chain(1, 11, 11, "11")
            rhs2_chain(1, 22, 10, "12")
            rhs2_chain(1, 32, 10, "13")
            finalize(0)
            contraction(1)
            finalize(1)

            # ---- fused LN2 for both timestamps (sel2b carries 1/2048)
            psd = psdep.tile([2 * COMP_DIM, 2], f32, tag="psDE")
            nc.tensor.matmul(psd[:], cat2F[:], selT, start=True, stop=True)
            sD = sp.tile([2 * COMP_DIM, 2], f32, tag="sD")
            nc.vector.tensor_copy(sD[:], psd[:])
            pse = psdep.tile([2, 2], f32, tag="psDE")
            nc.tensor.matmul(pse[:], sD[:], sel2b, start=True, stop=True)
            sE = sp.tile([2, 2], f32, tag="sE")
            nc.vector.tensor_copy(sE[:], pse[:])
            psf = psdep.tile([128, 2], f32, tag="psDE")
            nc.tensor.matmul(psf[:], bcast2, sE[:], start=True, stop=True)
            bS = sp.tile([128, 2], f32, tag="bS")
            nc.vector.tensor_copy(bS[:], psf[:])

            mu = bS[:, 0:1]
            mu2 = sp.tile([128, 1], f32, tag="mu2")
            nc.vector.tensor_mul(mu2[:], bS[:, 0:1], bS[:, 0:1])
            ex2 = sp.tile([128, 1], f32, tag="ex2")
            nc.vector.tensor_sub(ex2[:], bS[:, 1:2], mu2[:])
            sd2 = sp.tile([128, 1], f32, tag="sd2")
            nc.scalar.activation(sd2[:], ex2[:],
                                 mybir.ActivationFunctionType.Sqrt,
                                 bias=epsb)
            rr = sp.tile([128, 1], f32, tag="rr")
            nc.vector.reciprocal(rr[:], sd2[:])

            obuf = sp.tile([128, COMP_DIM], f32, tag="obuf")
            nc.vector.tensor_scalar(obuf[:], cat2F[:, 0:COMP_DIM],
                                    mu, rr[:],
                                    mybir.AluOpType.subtract,
                                    mybir.AluOpType.mult)
            nc.vector.tensor_mul(obuf[:], obuf[:], g2)
            nc.vector.tensor_add(obuf[:], obuf[:], b2)

            nc.sync.dma_start(out_d.ap().rearrange("t j c -> (t j) c"), obuf[:])

    nc.compile()
    return nc


def _get_program():
    global _PROGRAM
    if _PROGRAM is None:
        _PROGRAM = _build_program()
    return _PROGRAM


def _prepare_inputs(x, ln1_g, ln1_b, ln2_g, ln2_b, node_idx, stacked_indices):
    """Host-side index preprocessing + weight prep. Returns list of in_maps."""
    node_idx = np.asarray(node_idx).astype(np.int64)
    stacked = np.asarray(stacked_indices).astype(np.int64)
    x = np.asarray(x, dtype=np.float32)
    ln1_g = np.asarray(ln1_g, dtype=np.float32)
    ln1_b = np.asarray(ln1_b, dtype=np.float32)
    ln2_g = np.asarray(ln2_g, dtype=np.float32)
    ln2_b = np.asarray(ln2_b, dtype=np.float32)

    # histogram bt[n, j] = count of node n in basket j  (index preprocessing)
    bt = np.zeros((NUM_NODES, COMP_LEN), dtype=np.float32)
    j_ids = np.broadcast_to(np.arange(COMP_LEN)[:, None], stacked.shape)
    np.add.at(bt, (stacked.ravel(), j_ids.ravel()), 1.0)
    node_used = bt.any(axis=1)

    # weight prep: [Wg(32) | 1 | 0*15 | ssqsel(16, col 49 = 1)]
    wstat = np.zeros((E, 64), dtype=np.float32)
    wstat[np.arange(E), np.arange(E) // 4] = ln1_g
    wstat[:, 32] = 1.0
    wstat[:, 49] = 1.0
    wstat_bf = wstat.astype(BF16)
    scv = ln1_g.reshape(COMP_DIM, 4).sum(1)
    bbv = ln1_b.reshape(COMP_DIM, 4).mean(1)
    # sc is used against lambda which already carries 1/max_len (via R4S)
    sc782 = np.broadcast_to(-scv, (COMP_LEN, COMP_DIM))
    bb782 = np.broadcast_to(bbv / MAX_LEN, (COMP_LEN, COMP_DIM))
    g2 = ln2_g.reshape(COMP_LEN, COMP_DIM)
    b2 = ln2_b.reshape(COMP_LEN, COMP_DIM)
    cst4 = np.tile(
        np.concatenate([sc782, bb782, g2, b2], axis=1).astype(np.float32),
        (2, 1))

    cmerge = np.zeros((128, 262), dtype=np.float32)
    cmerge[:, 0:128] = cst4
    cmerge[0:COMP_LEN, 128] = 1.0           # selT col 0
    cmerge[COMP_LEN:128, 129] = 1.0         # selT col 1
    cmerge[:, 130] = EPS                    # LN2 sqrt bias
    cmerge[:, 131] = EPS * SDS              # folded LN1 sqrt bias
    cmerge[0:COMP_DIM, 132] = 1.0 / 2048.0  # sel2b col 0
    cmerge[COMP_DIM:COMP_LEN, 133] = 1.0 / 2048.0
    cmerge[0, 134:134 + COMP_LEN] = 1.0     # bc2 row 0
    cmerge[1, 134 + COMP_LEN:262] = 1.0     # bc2 row 1

    in_maps = []
    for core in range(N_CORES):
        ts = list(range(core * T_LOC, (core + 1) * T_LOC))
        am = np.zeros((T_LOC, 128, CH, COMP_LEN),
                      dtype=ml_dtypes.float8_e4m3)
        xbt = np.empty((T_LOC, E, NT), dtype=BF16)
        for ti, tg in enumerate(ts):
            nt_ids = node_idx[tg, :N_NODE]
            kept = np.flatnonzero(node_used[nt_ids])
            if len(kept) > NT:
                print(f"WARNING: kept token overflow {len(kept)} > {NT}",
                      file=sys.stderr)
                kept = kept[:NT]
            nk = len(kept)
            sel = np.zeros(NT, dtype=np.int64)
            sel[:nk] = kept
            xbt[ti] = np.ascontiguousarray(x[tg, sel, :].T).astype(BF16)
            a_full = bt[nt_ids[sel], :]
            a_full[nk:, :] = 0.0
            am[ti] = a_full.reshape(CH, 128, COMP_LEN).transpose(1, 0, 2)
        in_maps.append({
            "xbt": xbt,
            "am": am,
            "wstat": wstat_bf,
            "cmerge": cmerge,
        })
    return in_maps


def kernel(x, ln1_g, ln1_b, ln2_g, ln2_b, node_idx, stacked_indices,
           n_node=N_NODE, num_nodes=NUM_NODES):
    global LAST_RESULTS
    from concourse.bass_utils import run_bass_kernel_spmd

    nc = _get_program()
    in_maps = _prepare_inputs(x, ln1_g, ln1_b, ln2_g, ln2_b, node_idx,
                              stacked_indices)

    if os.environ.get("KERNEL_SIM"):
        outs = _run_sim(nc, in_maps)
    else:
        res = run_bass_kernel_spmd(
            nc, in_maps, core_ids=list(range(N_CORES)),
            trace=bool(os.environ.get("KERNEL_TRACE")),
        )
        LAST_RESULTS = res
        outs = [r["out"] for r in res.results]

    full = np.concatenate(outs, axis=0)           # [16, 64, 32]
    return full.reshape(T, 1, COMP_LEN * COMP_DIM).astype(np.float32)


def _run_sim(nc, in_maps):
    """CoreSim path (KERNEL_SIM=1): simulate cores serially."""
    from concourse.bass_interp import CoreSim
    outs = []
    ncores = int(os.environ.get("KERNEL_SIM_CORES", "1"))
    for core, im in enumerate(in_maps[:ncores]):
        sim = CoreSim(nc, trace=False)
        for k, v in im.items():
            sim.tensor(k)[:] = v
        sim.simulate(check_with_hw=False)
        outs.append(np.array(sim.tensor("out")))
    for core in range(ncores, len(in_maps)):
        outs.append(np.zeros((T_LOC, COMP_LEN, COMP_DIM), np.float32))
    return outs
